# revision 8
# baseline (speedup 1.0000x reference)
"""GPTQ 4-bit fused dequant + GEMM + bias + residual for Trainium2 (Bass/Tile).

Problem: out[b,s,n] = sum_k x[b,s,k] * W[k,n] + bias[n] + residual[b,s,n]
  where W = (q - z) * s is 4-bit group-quantized (group size 128 along K),
  x: [4, 2048, 4096] f32, packed weight: [512, 4096] int32 (8 nibbles/word).

Sharding: data-parallel over rows (B*S = 8192 -> 1024 rows/core on 8 cores).
Each core reads its x/residual shard plus the (small, packed) full weight,
dequantizes W on-chip, and computes its output shard; no collectives.

The kernel is TensorE-bound: the bf16 GEMM floor is ~437 us/core and the
schedule keeps the PE array back-to-back (measured ~218 ns per 128x128x512
matmul vs the N/2.4GHz streaming floor). fp8 DoubleRow was tried and
rejected: quarter-K fp8 passes accuracy (1.67e-2 < 2e-2; more fp8 fails)
but its power draw clock-throttles the whole chip by 1.2x, a net loss.

Host prep does all layout work so the device only streams:
  - x transposed/permuted/bf16-cast to [p, t, m] with k = 1024a + 8p + j for
    t = 8s + 4h + a (j = s + 4h), making the packed-word unpacking full-width
    with both matmul operands on the same k ordering; no on-chip transpose.
  - packed weights pre-split into u16 halves, chunk-major (fully contiguous
    per-chunk loads); scales/zeros stay COMPACT (8 group-rows per a-block)
    and are partition-broadcast by the DMA itself (stride-0 source AP), a
    16x traffic cut that takes the chunk-0 head off the DMA-bandwidth wall.
  - bias folded into residual (exact f32 add).

Scheduling (only SP/sync and Activation/scalar have HW DMA rings; a ring
holds ~5 outstanding descriptors, first completion ~5.5 us after issue,
then ~1 us per 128 KB): chunk-0's gating loads are split per-a across both
rings in consumption order (sync: weight pieces + compact zeros/scales;
scalar: x pieces), the s=0 dequant runs in per-a pieces, and chunk-0's
s=0 matmuls go a-outer so the first 8 matmuls only need the first pieces.
A memset-fed stream of N=512 warm-up matmuls holds the PE busy from the
preamble so the HAM clock-gate is at 8/8 (2.4 GHz) before the first real
matmul. Bulk x / later chunks / residual are ordered by first-use deadline;
residual loads go mid-chunk on the scalar ring; j-outer/mt-inner matmul
order staggers PSUM bank release at chunk boundaries, with the per-mt
epilogue interleaved into the final sweep and the last tile's epilogue
split 4-ways across both rings.
"""

import numpy as np

import concourse.mybir as mybir
import concourse.tile as tile
from concourse import bacc
from concourse.bass_utils import run_bass_kernel_spmd

F32 = mybir.dt.float32
BF16 = mybir.dt.bfloat16
I32 = mybir.dt.int32
U16 = mybir.dt.uint16

P = 128  # partitions
JT = 8  # nibbles per int32
NIB = 4  # bits per nibble

# Full problem shape (hardcoded per harness contract)
B, S, K, N = 4, 2048, 4096, 4096
N_CORES = 8
M_FULL = B * S
M_SHARD = M_FULL // N_CORES


def host_prep(input, weight, weight_scales, weight_zeros, bias, residual,
              n=N, nc_chunk=512):
    """Host-side layout transforms (device streams these directly)."""
    import ml_dtypes

    BF = ml_dtypes.bfloat16
    A = (K // JT) // P  # 4
    NCH = n // nc_chunk

    # x[m, 1024a + 8p + j] -> xtp[p, 8s + 4h + a, m], j = s + 4h, bf16
    xf = np.asarray(input, dtype=np.float32).reshape(M_FULL, K)
    x5 = xf.reshape(M_FULL, A, P, 2, 4)
    xtp = x5.astype(BF).transpose(2, 4, 3, 1, 0)
    xtp = np.ascontiguousarray(xtp.reshape(P, JT * A, M_FULL))

    # packed words -> u16 halves, chunk-major: whx[ci, h, p, a, nc]
    w = np.ascontiguousarray(np.asarray(weight, dtype=np.int32))
    wsx = w.view("<u2").reshape(A, P, n, 2).transpose(3, 1, 0, 2)
    wsx = wsx.reshape(2, P, A, NCH, nc_chunk).transpose(3, 0, 1, 2, 4)
    whx = np.ascontiguousarray(wsx)  # [NCH, 2, P, A, nc]

    # compact scales/zeros: zcx[ci, j, a, nc] = z[8a + j, n-slice]; the DMA
    # partition-broadcasts row j to partitions 16j..16j+15
    G = weight_scales.shape[0]
    AG = G // JT

    def compact(t):
        r = np.asarray(t, dtype=np.float32).reshape(AG, JT, NCH, nc_chunk)
        r = r.transpose(2, 1, 0, 3)  # [NCH, JT, AG, nc]
        return np.ascontiguousarray(r.astype(BF))

    zcx = compact(weight_zeros)
    scx = compact(weight_scales)

    # bias folded into residual (exact f32 add)
    res = np.asarray(residual, dtype=np.float32).reshape(M_FULL, n)
    res = res + np.asarray(bias, dtype=np.float32)[None, :]

    return xtp, whx, zcx, scx, np.ascontiguousarray(res)


def build_nc(m_shard=M_SHARD, k=K, n=N, nc_chunk=512):
    """Build the per-core Bass program (SPMD: same program on all cores)."""
    KP = k // JT  # packed rows (512)
    A = KP // P  # 128-row blocks of packed rows (4)
    MT = m_shard // P  # m tiles (8)
    NCH = n // nc_chunk  # n chunks (8)

    nc = bacc.Bacc("TRN2", target_bir_lowering=False)

    xtp = nc.dram_tensor("xtp", [P, JT * A, m_shard], BF16, kind="ExternalInput")
    whx = nc.dram_tensor("whx", [NCH, 2, P, A, nc_chunk], U16, kind="ExternalInput")
    zcx = nc.dram_tensor("zcx", [NCH, JT, A, nc_chunk], BF16, kind="ExternalInput")
    scx = nc.dram_tensor("scx", [NCH, JT, A, nc_chunk], BF16, kind="ExternalInput")
    res_in = nc.dram_tensor("res", [m_shard, n], F32, kind="ExternalInput")
    out = nc.dram_tensor("out", [m_shard, n], F32, kind="ExternalOutput")

    def bc(src):  # compact [JT, A, nc] -> partition-broadcast [128, A, nc]
        return src.unsqueeze(1).broadcast_to((JT, P // JT, A, nc_chunk))

    with tile.TileContext(nc) as tc:
        with (
            tc.tile_pool(name="persist", bufs=1) as persist,
            tc.tile_pool(name="ws", bufs=3) as ws_pool,
            tc.tile_pool(name="qs", bufs=4) as qs_pool,
            tc.tile_pool(name="q", bufs=6) as q_pool,
            tc.tile_pool(name="zs", bufs=3) as zs_pool,
            tc.tile_pool(name="res", bufs=12) as res_pool,
            tc.tile_pool(name="osb", bufs=3) as osb_pool,
            tc.tile_pool(name="psum", bufs=8, space="PSUM") as psum_pool,
        ):
            def load_chunk(ci):
                w0 = ws_pool.tile([P, A, nc_chunk], U16, tag="ws0",
                                  name=f"ws{ci}_0")
                nc.sync.dma_start(w0[:], whx[ci][0])
                zb = zs_pool.tile([P, A, nc_chunk], BF16, tag="zb",
                                  name=f"zb{ci}")
                sb = zs_pool.tile([P, A, nc_chunk], BF16, tag="sb",
                                  name=f"sb{ci}")
                nc.sync.dma_start(zb[:], bc(zcx[ci]))
                nc.sync.dma_start(sb[:], bc(scx[ci]))
                w1 = ws_pool.tile([P, A, nc_chunk], U16, tag="ws1",
                                  name=f"ws{ci}_1")
                nc.sync.dma_start(w1[:], whx[ci][1])
                return w0, w1, zb, sb

            xTs = [
                persist.tile([P, A, m_shard], BF16, tag=f"xT{i}", name=f"xT{i}")
                for i in range(JT)
            ]

            # ---- HAM pre-warm: memset-fed N=512 matmuls keep the PE busy
            # from the preamble through the DMA head so the clock-gate is
            # at 8/8 before the first real matmul; no DMA dependency ----
            warm = persist.tile([P, 544], BF16, tag="warm", name="warm")
            nc.vector.memset(warm[:], 0.5)
            ps_warm = psum_pool.tile([P, nc_chunk], F32, tag="ps", name="ps_warm")
            for i in range(18):
                nc.tensor.matmul(
                    ps_warm[:32, :], warm[:, :32], warm[:, 32:],
                    start=True, stop=True,
                )

            # ---- chunk-0 gating loads, split per-a across both HW rings in
            # consumption order ----
            w00 = ws_pool.tile([P, A, nc_chunk], U16, tag="ws0", name="ws0_0")
            w01 = ws_pool.tile([P, A, nc_chunk], U16, tag="ws1", name="ws0_1")
            zb0 = zs_pool.tile([P, A, nc_chunk], BF16, tag="zb", name="zb0")
            sb0 = zs_pool.tile([P, A, nc_chunk], BF16, tag="sb", name="sb0")
            nc.sync.dma_start(w00[:, 0, :], whx[0][0][:, 0])
            nc.sync.dma_start(zb0[:], bc(zcx[0]))
            nc.sync.dma_start(sb0[:], bc(scx[0]))
            for a in range(1, A):
                nc.sync.dma_start(w00[:, a, :], whx[0][0][:, a])
            for a in range(A):
                nc.sync.dma_start(w01[:, a, :], whx[0][1][:, a])
            for a in range(A):
                nc.scalar.dma_start(xTs[0][:, a, :], xtp[:, a : a + 1, :])
            for a in range(A):
                nc.scalar.dma_start(xTs[1][:, a, :], xtp[:, A + a : A + a + 1, :])
            # bulk x / chunk-1 weights / residual ordered by first-use
            # deadline (sync also carries per-chunk out stores in the loop)
            nc.sync.dma_start(xTs[2][:], xtp[:, 2 * A : 3 * A, :])
            nc.sync.dma_start(xTs[4][:], xtp[:, 4 * A : 5 * A, :])
            nc.scalar.dma_start(xTs[3][:], xtp[:, 3 * A : 4 * A, :])
            nc.scalar.dma_start(xTs[5][:], xtp[:, 5 * A : 6 * A, :])
            chunks = {0: (w00, w01, zb0, sb0), 1: load_chunk(1)}
            nc.sync.dma_start(xTs[6][:], xtp[:, 6 * A : 7 * A, :])
            nc.scalar.dma_start(xTs[7][:], xtp[:, 7 * A : 8 * A, :])

            def deq(w0, w1, zb, sb, s, ci):
                # ((word >> 4s) & 15) per u16 half (the sub below casts
                # u16 -> bf16; bitwise TS ops cannot cast)
                qjs = []
                for h, wsh in ((0, w0), (1, w1)):
                    qsb = qs_pool.tile([P, A, nc_chunk], U16, tag="qs",
                                       name=f"qs{ci}_{s}_{h}")
                    nc.vector.tensor_scalar(
                        out=qsb[:],
                        in0=wsh[:],
                        scalar1=NIB * s,
                        scalar2=15,
                        op0=mybir.AluOpType.logical_shift_right,
                        op1=mybir.AluOpType.bitwise_and,
                    )
                    qj = q_pool.tile([P, A, nc_chunk], BF16, tag="q",
                                     name=f"q{ci}_{s}_{h}")
                    nc.vector.tensor_sub(qj[:], qsb[:], zb[:])
                    nc.vector.tensor_mul(qj[:], qj[:], sb[:])
                    qjs.append(qj)
                return qjs

            # chunk-0 s=0 dequant in per-a pieces: each piece only waits on
            # its own loads, so the first matmul's rhs is ready ~1.5 us
            # after the first weight piece lands
            deq0 = []
            for h, wsh in ((0, w00), (1, w01)):
                qsb = qs_pool.tile([P, A, nc_chunk], U16, tag="qs",
                                   name=f"qs0_0_{h}")
                qj = q_pool.tile([P, A, nc_chunk], BF16, tag="q",
                                 name=f"q0_0_{h}")
                for a in range(A):
                    nc.vector.tensor_scalar(
                        out=qsb[:, a, :],
                        in0=wsh[:, a, :],
                        scalar1=0,
                        scalar2=15,
                        op0=mybir.AluOpType.logical_shift_right,
                        op1=mybir.AluOpType.bitwise_and,
                    )
                    nc.vector.tensor_sub(qj[:, a, :], qsb[:, a, :], zb0[:, a, :])
                    nc.vector.tensor_mul(qj[:, a, :], qj[:, a, :], sb0[:, a, :])
                deq0.append(qj)

            for ci in range(NCH):
                nsl = slice(ci * nc_chunk, (ci + 1) * nc_chunk)
                w0, w1, zb, sb = chunks.pop(ci)
                if ci + 2 < NCH:
                    chunks[ci + 2] = load_chunk(ci + 2)

                ps = [
                    psum_pool.tile([P, nc_chunk], F32, tag="ps", name=f"ps{ci}_{mt}")
                    for mt in range(MT)
                ]
                res_tiles = []

                for s in range(3):
                    qjs = deq0 if s == 0 else deq(w0, w1, zb, sb, s, ci)
                    if ci == 0 and s == 0:
                        # a-outer: the first 8 matmuls only need the a=0
                        # pieces; later pieces stream in behind them
                        for h in range(2):
                            for a in range(A):
                                for mt in range(MT):
                                    nc.tensor.matmul(
                                        ps[mt][:],
                                        xTs[h][:, a, mt * P : (mt + 1) * P],
                                        qjs[h][:, a, :],
                                        start=(h == 0 and a == 0),
                                        stop=False,
                                    )
                    else:
                        for h in range(2):
                            for mt in range(MT):
                                for a in range(A):
                                    nc.tensor.matmul(
                                        ps[mt][:],
                                        xTs[2 * s + h][:, a, mt * P : (mt + 1) * P],
                                        qjs[h][:, a, :],
                                        start=(s == 0 and h == 0 and a == 0),
                                        stop=False,
                                    )
                    if s >= 1:
                        # residual loads mid-chunk (scalar ring): off the
                        # head/boundary critical path, ready for the epilogue
                        for mt in range((s - 1) * MT // 2, s * MT // 2):
                            r = res_pool.tile([P, nc_chunk], F32, tag="res",
                                              name=f"res{ci}_{mt}")
                            nc.scalar.dma_start(
                                r[:], res_in[mt * P : (mt + 1) * P, nsl]
                            )
                            res_tiles.append(r)

                # last k-group: dequant, then next chunk's first dequant
                # (ahead of the epilogue adds in the DVE queue), then matmuls
                # with the per-mt epilogue interleaved at each mt's stop
                qjs = deq(w0, w1, zb, sb, 3, ci)
                if ci + 1 < NCH:
                    deq0 = deq(*chunks[ci + 1], 0, ci + 1)

                # mt-outer: each mt's last 8 k-tiles run consecutively, so
                # its stop lands up to ~12us before chunk end and the
                # epilogue (ADD + store) drains while later mts compute
                for mt in range(MT):
                    for h in range(2):
                        for a in range(A):
                            nc.tensor.matmul(
                                ps[mt][:],
                                xTs[6 + h][:, a, mt * P : (mt + 1) * P],
                                qjs[h][:, a, :],
                                start=False,
                                stop=(h == 1 and a == A - 1),
                            )
                    osb = osb_pool.tile([P, nc_chunk], F32, tag="osb")
                    if ci == NCH - 1 and mt == MT - 1:
                        # exposed tail: split the last tile's epilogue into
                        # 4 pieces across both HW rings so add+store pipeline
                        for pi in range(4):
                            cs = slice(pi * 128, (pi + 1) * 128)
                            nc.vector.tensor_add(
                                osb[:, cs], ps[mt][:, cs], res_tiles[mt][:, cs]
                            )
                            eng = nc.sync if pi % 2 == 0 else nc.scalar
                            eng.dma_start(
                                out[
                                    mt * P : (mt + 1) * P,
                                    ci * nc_chunk + pi * 128 : ci * nc_chunk
                                    + (pi + 1) * 128,
                                ],
                                osb[:, cs],
                            )
                    else:
                        nc.vector.tensor_add(osb[:], ps[mt][:], res_tiles[mt][:])
                        nc.sync.dma_start(out[mt * P : (mt + 1) * P, nsl], osb[:])

    nc.compile()
    return nc


_NC_CACHE = {}


def _get_nc():
    if "nc" not in _NC_CACHE:
        _NC_CACHE["nc"] = build_nc()
    return _NC_CACHE["nc"]


def kernel(input, weight, weight_scales, weight_zeros, bias, residual, **run_kwargs):
    """Full-input entry point: shards across 8 NeuronCores, returns full output."""
    xtp, whx, zcx, scx, res = host_prep(
        input, weight, weight_scales, weight_zeros, bias, residual
    )
    nc = _get_nc()
    in_maps = []
    for i in range(N_CORES):
        rows = slice(i * M_SHARD, (i + 1) * M_SHARD)
        in_maps.append(
            {
                "xtp": np.ascontiguousarray(xtp[:, :, rows]),
                "whx": whx,
                "zcx": zcx,
                "scx": scx,
                "res": np.ascontiguousarray(res[rows]),
            }
        )
    result = run_bass_kernel_spmd(
        nc, in_maps, core_ids=list(range(N_CORES)), **run_kwargs
    )
    shards = [result.results[i]["out"] for i in range(N_CORES)]
    full = np.concatenate(shards, axis=0).reshape(B, S, N).astype(np.float32)
    if run_kwargs:
        return full, result
    return full


# revision 9
# speedup vs baseline: 1.0506x; 1.0506x over previous
"""GPTQ 4-bit fused dequant + GEMM + bias + residual for Trainium2 (Bass/Tile).

Problem: out[b,s,n] = sum_k x[b,s,k] * W[k,n] + bias[n] + residual[b,s,n]
  where W = (q - z) * s is 4-bit group-quantized (group size 128 along K),
  x: [4, 2048, 4096] f32, packed weight: [512, 4096] int32 (8 nibbles/word).

Sharding: data-parallel over rows (B*S = 8192 -> 1024 rows/core on 8 cores).
Each core reads its x/residual shard plus the (small, packed) full weight,
dequantizes W on-chip, and computes its output shard; no collectives.

The kernel is TensorE-bound: the bf16 GEMM floor is ~437 us/core and the
schedule keeps the PE array back-to-back (measured ~218 ns per 128x128x512
matmul vs the N/2.4GHz streaming floor). fp8 DoubleRow was tried and
rejected: quarter-K fp8 passes accuracy (1.67e-2 < 2e-2; more fp8 fails)
but its power draw clock-throttles the whole chip by 1.2x, a net loss.
A compact-scales/zeros load via stride-0 (partition-broadcast) DMA source
was tried and rejected: the broadcast pattern transfers ~10x slower than a
contiguous load of the host-prebroadcast equivalent.

Host prep does all layout work so the device only streams:
  - x transposed/permuted/bf16-cast to [p, t, m] with k = 1024a + 8p + j for
    t = 8s + 4h + a (j = s + 4h), making the packed-word unpacking full-width
    with both matmul operands on the same k ordering; no on-chip transpose.
  - packed-weight h0 halves + zeros + scales packed into ONE chunk-major
    tensor (hdx) so a chunk's whole dequant input is a single descriptor
    (the bf16 zeros/scales ride as u16 bits, bitcast on-chip); h1 halves
    separate (wsx1) so the h0 dequant only waits on half the bytes.
  - bias folded into the residual (exact f32 add).

Scheduling (only SP/sync and Activation/scalar have HW DMA rings; a ring
holds ~5 outstanding descriptors, first completion ~5.5 us after issue,
then ~one per 1-3 us): chunk-0's gating loads are split per-a and spread
across both rings in consumption order (sync: packed ws/zeros/scales
pieces then ws-h1 pieces; scalar: x pieces), the s=0 dequant runs in
per-a pieces, and chunk-0's s=0 matmuls go a-outer so the first 8 matmuls
only need the first pieces. A memset-fed stream of N=512 warm-up matmuls
holds the PE busy from the preamble through the DMA head so the HAM
clock-gate is at 8/8 (2.4 GHz) before the first real matmul. Bulk x /
later chunks / residual are ordered by first-use deadline; residual loads
go mid-chunk on the scalar ring; j-outer/mt-inner matmul order staggers
PSUM bank release at chunk boundaries, with the per-mt epilogue
interleaved into the final sweep and the last tile's epilogue split
4-ways across both rings.
"""

import numpy as np

import concourse.mybir as mybir
import concourse.tile as tile
from concourse import bacc
from concourse.bass_utils import run_bass_kernel_spmd

F32 = mybir.dt.float32
BF16 = mybir.dt.bfloat16
I32 = mybir.dt.int32
U16 = mybir.dt.uint16

P = 128  # partitions
JT = 8  # nibbles per int32
NIB = 4  # bits per nibble

# Full problem shape (hardcoded per harness contract)
B, S, K, N = 4, 2048, 4096, 4096
N_CORES = 8
M_FULL = B * S
M_SHARD = M_FULL // N_CORES


def host_prep(input, weight, weight_scales, weight_zeros, bias, residual,
              n=N, nc_chunk=512):
    """Host-side layout transforms (device streams these directly)."""
    import ml_dtypes

    BF = ml_dtypes.bfloat16
    A = (K // JT) // P  # 4
    NCH = n // nc_chunk

    # x[m, 1024a + 8p + j] -> xtp[p, 8s + 4h + a, m], j = s + 4h, bf16
    xf = np.asarray(input, dtype=np.float32).reshape(M_FULL, K)
    x5 = xf.reshape(M_FULL, A, P, 2, 4)
    xtp = x5.astype(BF).transpose(2, 4, 3, 1, 0)
    xtp = np.ascontiguousarray(xtp.reshape(P, JT * A, M_FULL))

    # packed words -> u16 halves, chunk-major: whx[h][ci, p, a, nc]
    w = np.ascontiguousarray(np.asarray(weight, dtype=np.int32))
    wsx = w.view("<u2").reshape(A, P, n, 2).transpose(1, 3, 0, 2)
    wsx = wsx.reshape(P, 2, A, NCH, nc_chunk).transpose(3, 1, 0, 2, 4)
    wh0 = np.ascontiguousarray(wsx[:, 0])  # [NCH, P, A, nc]
    wh1 = np.ascontiguousarray(wsx[:, 1])

    # scales/zeros broadcast to [ci, p, a, nc]: zb[p, a, n] = z[8a + p//16, n]
    G = weight_scales.shape[0]
    AG = G // JT

    def bcast(t):
        r = t.reshape(AG, JT, n)
        r = np.repeat(r, 16, axis=1)
        r = r.transpose(1, 0, 2)
        r = r.reshape(P, AG, NCH, nc_chunk).transpose(2, 0, 1, 3)
        return np.ascontiguousarray(r.astype(BF))

    zbx = bcast(np.asarray(weight_zeros, dtype=np.float32))
    sbx = bcast(np.asarray(weight_scales, dtype=np.float32))

    # pack [ws_h0 | zeros | scales] along a new axis so one descriptor per
    # chunk (or per a-piece for chunk 0) carries the whole dequant input
    hdx = np.stack(
        [wh0, zbx.view("<u2"), sbx.view("<u2")], axis=3
    )  # [NCH, P, A, 3, nc] u16
    hdx = np.ascontiguousarray(hdx)

    # bias folded into residual (exact f32 add)
    res = np.asarray(residual, dtype=np.float32).reshape(M_FULL, n)
    res = res + np.asarray(bias, dtype=np.float32)[None, :]

    return xtp, hdx, np.ascontiguousarray(wh1), np.ascontiguousarray(res)


def build_nc(m_shard=M_SHARD, k=K, n=N, nc_chunk=512):
    """Build the per-core Bass program (SPMD: same program on all cores)."""
    KP = k // JT  # packed rows (512)
    A = KP // P  # 128-row blocks of packed rows (4)
    MT = m_shard // P  # m tiles (8)
    NCH = n // nc_chunk  # n chunks (8)

    nc = bacc.Bacc("TRN2", target_bir_lowering=False)

    xtp = nc.dram_tensor("xtp", [P, JT * A, m_shard], BF16, kind="ExternalInput")
    hdx = nc.dram_tensor("hdx", [NCH, P, A, 3, nc_chunk], U16, kind="ExternalInput")
    wsx1 = nc.dram_tensor("wsx1", [NCH, P, A, nc_chunk], U16, kind="ExternalInput")
    res_in = nc.dram_tensor("res", [m_shard, n], F32, kind="ExternalInput")
    out = nc.dram_tensor("out", [m_shard, n], F32, kind="ExternalOutput")

    with tile.TileContext(nc) as tc:
        with (
            tc.tile_pool(name="persist", bufs=1) as persist,
            tc.tile_pool(name="hd", bufs=3) as hd_pool,
            tc.tile_pool(name="ws", bufs=3) as ws_pool,
            tc.tile_pool(name="qs", bufs=4) as qs_pool,
            tc.tile_pool(name="q", bufs=6) as q_pool,
            tc.tile_pool(name="res", bufs=12) as res_pool,
            tc.tile_pool(name="osb", bufs=3) as osb_pool,
            tc.tile_pool(name="psum", bufs=8, space="PSUM") as psum_pool,
        ):
            def load_chunk(ci):
                hd = hd_pool.tile([P, A, 3, nc_chunk], U16, tag="hd",
                                  name=f"hd{ci}")
                nc.sync.dma_start(hd[:], hdx[ci])
                w1 = ws_pool.tile([P, A, nc_chunk], U16, tag="ws1",
                                  name=f"ws{ci}_1")
                nc.sync.dma_start(w1[:], wsx1[ci])
                return hd, w1

            xTs = [
                persist.tile([P, A, m_shard], BF16, tag=f"xT{i}", name=f"xT{i}")
                for i in range(JT)
            ]

            # ---- HAM pre-warm: memset-fed N=512 matmuls keep the PE busy
            # from the preamble through the DMA head so the clock-gate is
            # at 8/8 before the first real matmul; no DMA dependency ----
            warm = persist.tile([P, 544], BF16, tag="warm", name="warm")
            nc.vector.memset(warm[:], 0.5)
            ps_warm = psum_pool.tile([P, nc_chunk], F32, tag="ps", name="ps_warm")
            for i in range(18):
                nc.tensor.matmul(
                    ps_warm[:32, :], warm[:, :32], warm[:, 32:],
                    start=True, stop=True,
                )

            # ---- chunk-0 gating loads, split per-a across both HW rings in
            # consumption order ----
            hd0 = hd_pool.tile([P, A, 3, nc_chunk], U16, tag="hd", name="hd0")
            ws01 = ws_pool.tile([P, A, nc_chunk], U16, tag="ws1", name="ws0_1")
            for a in range(A):
                nc.sync.dma_start(hd0[:, a], hdx[0][:, a])
            for a in range(A):
                nc.sync.dma_start(ws01[:, a, :], wsx1[0][:, a])
            for a in range(A):
                nc.scalar.dma_start(xTs[0][:, a, :], xtp[:, a : a + 1, :])
            for a in range(A):
                nc.scalar.dma_start(xTs[1][:, a, :], xtp[:, A + a : A + a + 1, :])
            # bulk x / chunk-1 weights / residual ordered by first-use
            # deadline (sync also carries per-chunk out stores in the loop)
            nc.sync.dma_start(xTs[2][:], xtp[:, 2 * A : 3 * A, :])
            nc.scalar.dma_start(xTs[3][:], xtp[:, 3 * A : 4 * A, :])
            nc.sync.dma_start(xTs[4][:], xtp[:, 4 * A : 5 * A, :])
            nc.scalar.dma_start(xTs[5][:], xtp[:, 5 * A : 6 * A, :])
            chunks = {0: (hd0, ws01), 1: load_chunk(1)}
            nc.sync.dma_start(xTs[6][:], xtp[:, 6 * A : 7 * A, :])
            nc.scalar.dma_start(xTs[7][:], xtp[:, 7 * A : 8 * A, :])

            def deq(hd, w1, s, ci):
                # ((word >> 4s) & 15) per u16 half (the sub below casts
                # u16 -> bf16; bitwise TS ops cannot cast); zeros/scales are
                # bf16 bits riding in the packed hd tile
                zb = hd[:, :, 1, :].bitcast(BF16)
                sb = hd[:, :, 2, :].bitcast(BF16)
                qjs = []
                for h, wsh in ((0, hd[:, :, 0, :]), (1, w1[:])):
                    qsb = qs_pool.tile([P, A, nc_chunk], U16, tag="qs",
                                       name=f"qs{ci}_{s}_{h}")
                    nc.vector.tensor_scalar(
                        out=qsb[:],
                        in0=wsh,
                        scalar1=NIB * s,
                        scalar2=15,
                        op0=mybir.AluOpType.logical_shift_right,
                        op1=mybir.AluOpType.bitwise_and,
                    )
                    qj = q_pool.tile([P, A, nc_chunk], BF16, tag="q",
                                     name=f"q{ci}_{s}_{h}")
                    nc.vector.tensor_sub(qj[:], qsb[:], zb)
                    nc.vector.tensor_mul(qj[:], qj[:], sb)
                    qjs.append(qj)
                return qjs

            # chunk-0 s=0 dequant in per-a pieces: each piece only waits on
            # its own loads, so the first matmul's rhs is ready ~2 us after
            # the first packed piece lands
            zb0 = hd0[:, :, 1, :].bitcast(BF16)
            sb0 = hd0[:, :, 2, :].bitcast(BF16)
            deq0 = []
            for h, wsh in ((0, hd0[:, :, 0, :]), (1, ws01)):
                qsb = qs_pool.tile([P, A, nc_chunk], U16, tag="qs",
                                   name=f"qs0_0_{h}")
                qj = q_pool.tile([P, A, nc_chunk], BF16, tag="q",
                                 name=f"q0_0_{h}")
                for a in range(A):
                    nc.vector.tensor_scalar(
                        out=qsb[:, a, :],
                        in0=wsh[:, a, :],
                        scalar1=0,
                        scalar2=15,
                        op0=mybir.AluOpType.logical_shift_right,
                        op1=mybir.AluOpType.bitwise_and,
                    )
                    nc.vector.tensor_sub(qj[:, a, :], qsb[:, a, :], zb0[:, a, :])
                    nc.vector.tensor_mul(qj[:, a, :], qj[:, a, :], sb0[:, a, :])
                deq0.append(qj)

            for ci in range(NCH):
                nsl = slice(ci * nc_chunk, (ci + 1) * nc_chunk)
                hd, w1 = chunks.pop(ci)
                if ci + 2 < NCH:
                    chunks[ci + 2] = load_chunk(ci + 2)

                ps = [
                    psum_pool.tile([P, nc_chunk], F32, tag="ps", name=f"ps{ci}_{mt}")
                    for mt in range(MT)
                ]
                res_tiles = []

                for s in range(3):
                    qjs = deq0 if s == 0 else deq(hd, w1, s, ci)
                    if ci == 0 and s == 0:
                        # a-outer: the first 8 matmuls only need the a=0
                        # pieces; later pieces stream in behind them
                        for h in range(2):
                            for a in range(A):
                                for mt in range(MT):
                                    nc.tensor.matmul(
                                        ps[mt][:],
                                        xTs[h][:, a, mt * P : (mt + 1) * P],
                                        qjs[h][:, a, :],
                                        start=(h == 0 and a == 0),
                                        stop=False,
                                    )
                    else:
                        for h in range(2):
                            for mt in range(MT):
                                for a in range(A):
                                    nc.tensor.matmul(
                                        ps[mt][:],
                                        xTs[2 * s + h][:, a, mt * P : (mt + 1) * P],
                                        qjs[h][:, a, :],
                                        start=(s == 0 and h == 0 and a == 0),
                                        stop=False,
                                    )
                    if s >= 1:
                        # residual loads mid-chunk (scalar ring): off the
                        # head/boundary critical path, ready for the epilogue
                        for mt in range((s - 1) * MT // 2, s * MT // 2):
                            r = res_pool.tile([P, nc_chunk], F32, tag="res",
                                              name=f"res{ci}_{mt}")
                            nc.scalar.dma_start(
                                r[:], res_in[mt * P : (mt + 1) * P, nsl]
                            )
                            res_tiles.append(r)

                # last k-group: dequant, then next chunk's first dequant
                # (ahead of the epilogue adds in the DVE queue), then matmuls
                # with the per-mt epilogue interleaved at each mt's stop
                qjs = deq(hd, w1, 3, ci)
                if ci + 1 < NCH:
                    deq0 = deq(*chunks[ci + 1], 0, ci + 1)

                # mt-outer: each mt's last 8 k-tiles run consecutively, so
                # its stop lands up to ~12us before chunk end and the
                # epilogue (ADD + store) drains while later mts compute
                for mt in range(MT):
                    for h in range(2):
                        for a in range(A):
                            nc.tensor.matmul(
                                ps[mt][:],
                                xTs[6 + h][:, a, mt * P : (mt + 1) * P],
                                qjs[h][:, a, :],
                                start=False,
                                stop=(h == 1 and a == A - 1),
                            )
                    osb = osb_pool.tile([P, nc_chunk], F32, tag="osb")
                    if ci == NCH - 1 and mt == MT - 1:
                        # exposed tail: split the last tile's epilogue into
                        # 4 pieces across both HW rings so add+store pipeline
                        for pi in range(4):
                            cs = slice(pi * 128, (pi + 1) * 128)
                            nc.vector.tensor_add(
                                osb[:, cs], ps[mt][:, cs], res_tiles[mt][:, cs]
                            )
                            eng = nc.sync if pi % 2 == 0 else nc.scalar
                            eng.dma_start(
                                out[
                                    mt * P : (mt + 1) * P,
                                    ci * nc_chunk + pi * 128 : ci * nc_chunk
                                    + (pi + 1) * 128,
                                ],
                                osb[:, cs],
                            )
                    else:
                        nc.vector.tensor_add(osb[:], ps[mt][:], res_tiles[mt][:])
                        nc.sync.dma_start(out[mt * P : (mt + 1) * P, nsl], osb[:])

    nc.compile()
    return nc


_NC_CACHE = {}


def _get_nc():
    if "nc" not in _NC_CACHE:
        _NC_CACHE["nc"] = build_nc()
    return _NC_CACHE["nc"]


def kernel(input, weight, weight_scales, weight_zeros, bias, residual, **run_kwargs):
    """Full-input entry point: shards across 8 NeuronCores, returns full output."""
    xtp, hdx, wsx1, res = host_prep(
        input, weight, weight_scales, weight_zeros, bias, residual
    )
    nc = _get_nc()
    in_maps = []
    for i in range(N_CORES):
        rows = slice(i * M_SHARD, (i + 1) * M_SHARD)
        in_maps.append(
            {
                "xtp": np.ascontiguousarray(xtp[:, :, rows]),
                "hdx": hdx,
                "wsx1": wsx1,
                "res": np.ascontiguousarray(res[rows]),
            }
        )
    result = run_bass_kernel_spmd(
        nc, in_maps, core_ids=list(range(N_CORES)), **run_kwargs
    )
    shards = [result.results[i]["out"] for i in range(N_CORES)]
    full = np.concatenate(shards, axis=0).reshape(B, S, N).astype(np.float32)
    if run_kwargs:
        return full, result
    return full
